# revision 1
# baseline (speedup 1.0000x reference)
"""Trainium2 Bass kernel for nn_LoraLayer (grouped LoRA GEMM with token routing).

Strategy: data-parallel over tokens with replicated (packed) LoRA weights.
Each of the 8 cores takes an equal 1/8 share of every slot's tokens
(128/192/96/32 = 448 tokens), so all cores run an identical program on
identically-shaped data.

Device kernel (per core):
  GEMM1: inter^T[r, t] = A[s]^T-packed stationary passes (6 passes of 128
         columns each, module-packed, partition-aligned to 32) x streamed
         x^T token columns, accumulated over 16 h-chunks of 128.
  GEMM2: out[t, o] = inter (stationary, partition range matches GEMM1
         output rows) x streamed B columns (B host-packed into the same
         partition ranges), N=512 per matmul.
  PSUM -> SBUF copies alternate VectorE / ScalarE, staged per token block
  into a [T_b, 6144] f32 tile, then one DMA per token block to DRAM.

Host side: gather x rows by sorted_ids, build packed/swizzled weight
images, scatter per-core outputs back to token order.
"""

import contextlib
import ctypes
import os
import sys
import types

import ml_dtypes
import numpy as np

# ---------------------------------------------------------------- constants
BS, H, OUT = 4096, 2048, 2048
M, S, R = 3, 4, 64
SLOT_COUNTS = [1024, 1536, 768, 256]
SLOT_RANKS = [64, 32, 16, 48]
SLOT_OFFS = [0, 1024, 2560, 3328]
VALID = 3584
N_CORES = 8
CS8 = [c // N_CORES for c in SLOT_COUNTS]          # [128, 192, 96, 32]
TCORE = sum(CS8)                                   # 448
TOFF = [0, 128, 320, 416]                          # slot offsets within a core
KCH = H // 128                                     # 16 h-chunks

# GEMM1 passes: 6 stationary blocks of 128 A-pack columns each.
# Each entry: (slot, [(module, col_off_within_pass), ...]).  col_off is also
# the PSUM partition where that module's inter rows land (32-aligned).
PASSES = [
    (0, [(0, 0), (1, 64)]),
    (0, [(2, 0)]),
    (1, [(0, 0), (1, 32), (2, 64)]),
    (2, [(0, 0), (1, 32), (2, 64)]),
    (3, [(0, 0), (1, 64)]),
    (3, [(2, 0)]),
]
# (m, s) -> (pass_idx, partition_off)
LOC = {}
for _p, (_s, _mods) in enumerate(PASSES):
    for _m, _c in _mods:
        LOC[(_m, _s)] = (_p, _c)
# (m, s) -> B column group (6 groups of 2048 cols; partition ranges within a
# group are disjoint and equal to LOC partition ranges)
BGRP = {(0, 0): 0, (1, 0): 0, (2, 0): 1, (1, 3): 1,
        (0, 1): 2, (1, 1): 2, (2, 1): 2,
        (0, 2): 3, (1, 2): 3, (2, 2): 3,
        (0, 3): 4, (2, 3): 5}
NBG = 6
# token blocks (slot, tok_off_in_core, T_b) -- PSUM partition blocks <=128
TBLOCKS = [(0, 0, 128), (1, 128, 128), (1, 256, 64), (2, 320, 96), (3, 416, 32)]

BF16 = ml_dtypes.bfloat16

LAST_EXEC_NS = None

_nc_cache = {}


def _install_ntff_hook():
    """Agent image's antenv lacks axon_hooks; recreate it so trace=True works."""
    if "antenv.axon_hooks" in sys.modules:
        return True
    so_path = "/opt/axon/libaxon_pjrt.so"
    try:
        lib = ctypes.CDLL(so_path)
    except OSError:
        return False
    if not hasattr(lib, "axon_start_nrt_profile"):
        return False
    lib.axon_start_nrt_profile.argtypes = [ctypes.POINTER(ctypes.c_int64), ctypes.c_size_t]
    lib.axon_start_nrt_profile.restype = ctypes.c_int64
    lib.axon_stop_nrt_profile.argtypes = [ctypes.c_char_p]
    lib.axon_stop_nrt_profile.restype = ctypes.c_int64

    @contextlib.contextmanager
    def _hook(output_dir, device_ids):
        import jax
        jax.devices()
        if device_ids:
            ids = (ctypes.c_int64 * len(device_ids))(*device_ids)
            rc = lib.axon_start_nrt_profile(ids, len(device_ids))
        else:
            rc = lib.axon_start_nrt_profile(None, 0)
        if rc != 0:
            raise RuntimeError(f"axon_start_nrt_profile rc={rc}")
        try:
            yield
        finally:
            n = lib.axon_stop_nrt_profile(str(output_dir).encode())
            print(f"ntff profile: {n} file(s) -> {output_dir}", file=sys.stderr)

    mod = types.ModuleType("antenv.axon_hooks")
    mod.get_axon_ntff_profile_hook = lambda: _hook
    mod.set_axon_ntff_profile_hook = lambda h: None
    sys.modules["antenv.axon_hooks"] = mod
    return True


def _build_nc():
    import concourse.bass as bass  # noqa: F401
    import concourse.mybir as mybir
    from concourse import bacc
    from concourse.tile import TileContext

    dt_in = mybir.dt.bfloat16
    f32 = mybir.dt.float32

    nc = bacc.Bacc("TRN2", target_bir_lowering=False)
    xt = nc.dram_tensor("xt", [128, KCH * TCORE], dt_in, kind="ExternalInput")
    apk = nc.dram_tensor("apk", [128, len(PASSES) * KCH * 128], dt_in, kind="ExternalInput")
    bpk = nc.dram_tensor("bpk", [128, NBG * OUT], dt_in, kind="ExternalInput")
    y = nc.dram_tensor("y", [TCORE, M * OUT], f32, kind="ExternalOutput")

    with TileContext(nc) as tc:
        with (
            tc.tile_pool(name="w", bufs=1) as wpool,
            tc.tile_pool(name="interp", bufs=1) as ipool,
            tc.tile_pool(name="outp", bufs=3) as opool,
            tc.tile_pool(name="ps1", bufs=3, space="PSUM") as ps1,
            tc.tile_pool(name="ps2", bufs=4, space="PSUM") as ps2,
        ):
            xt_sb = wpool.tile([128, KCH * TCORE], dt_in, tag="xt")
            a_sb = wpool.tile([128, len(PASSES) * KCH * 128], dt_in, tag="a")
            b_sb = wpool.tile([128, NBG * OUT], dt_in, tag="b")

            # input DMAs: xt first (needed by every pass), then A per pass,
            # then B per column group.
            for q in range(4):
                c0, c1 = q * 4 * TCORE, (q + 1) * 4 * TCORE
                nc.sync.dma_start(xt_sb[:, c0:c1], xt[:, c0:c1])
            for p in range(len(PASSES)):
                c0, c1 = p * KCH * 128, (p + 1) * KCH * 128
                nc.sync.dma_start(a_sb[:, c0:c1], apk[:, c0:c1])
            for g in range(NBG):
                c0, c1 = g * OUT, (g + 1) * OUT
                nc.sync.dma_start(b_sb[:, c0:c1], bpk[:, c0:c1])

            inter = [None] * len(PASSES)
            copy_i = 0

            def gemm1(p):
                s, _mods = PASSES[p]
                T = CS8[s]
                pt = ps1.tile([128, T], f32, tag="ps1")
                for k in range(KCH):
                    nc.tensor.matmul(
                        pt[:, :T],
                        lhsT=a_sb[:, p * KCH * 128 + k * 128: p * KCH * 128 + (k + 1) * 128],
                        rhs=xt_sb[:, k * TCORE + TOFF[s]: k * TCORE + TOFF[s] + T],
                        start=(k == 0),
                        stop=(k == KCH - 1),
                    )
                it = ipool.tile([128, T], dt_in, tag=f"inter{p}")
                nc.vector.tensor_copy(it[:, :], pt[:, :T])
                inter[p] = it

            def gemm2(tb):
                nonlocal copy_i
                s, toff, Tb = tb
                r = SLOT_RANKS[s]
                ot = opool.tile([128, M * OUT], f32, tag="out")
                for m in range(M):
                    p, part = LOC[(m, s)]
                    g = BGRP[(m, s)]
                    lhs = inter[p][part:part + r, toff - TOFF[s]: toff - TOFF[s] + Tb]
                    for oc in range(4):
                        pt = ps2.tile([128, 512], f32, tag="ps2")
                        nc.tensor.matmul(
                            pt[:Tb, :],
                            lhsT=lhs,
                            rhs=b_sb[part:part + r, g * OUT + oc * 512: g * OUT + (oc + 1) * 512],
                            start=True,
                            stop=True,
                        )
                        eng = nc.vector if copy_i % 2 == 0 else nc.scalar
                        copy_i += 1
                        if eng is nc.vector:
                            eng.tensor_copy(ot[:Tb, m * OUT + oc * 512: m * OUT + (oc + 1) * 512], pt[:Tb, :])
                        else:
                            eng.copy(ot[:Tb, m * OUT + oc * 512: m * OUT + (oc + 1) * 512], pt[:Tb, :])
                nc.sync.dma_start(y[toff:toff + Tb, :], ot[:Tb, :])

            # interleave: slot's GEMM2 as soon as its passes are done
            gemm1(0)
            gemm1(1)
            gemm2(TBLOCKS[0])
            gemm1(2)
            gemm2(TBLOCKS[1])
            gemm2(TBLOCKS[2])
            gemm1(3)
            gemm2(TBLOCKS[3])
            gemm1(4)
            gemm1(5)
            gemm2(TBLOCKS[4])

    nc.finalize()
    return nc


def _get_nc():
    if "nc" not in _nc_cache:
        _nc_cache["nc"] = _build_nc()
    return _nc_cache["nc"]


def _host_reference(x, lora_A, lora_B, sorted_ids, row_slot, slot_ranks):
    """Numpy fallback (used only if runtime slot structure deviates from the
    compile-time layout)."""
    reordered = x[sorted_ids]
    out = np.zeros((BS, M * OUT), dtype=np.float32)
    for b in range(BS):
        s = row_slot[b]
        if s < 0:
            continue
        r = int(slot_ranks[s])
        row = reordered[b]
        for m in range(M):
            inter = row @ lora_A[m, s, :r, :].T
            out[sorted_ids[b], m * OUT:(m + 1) * OUT] = inter @ lora_B[m, s, :r, :]
    return out


def kernel(**inputs):
    global LAST_EXEC_NS
    x = np.asarray(inputs["x"], dtype=np.float32)
    lora_A = np.asarray(inputs["lora_A"], dtype=np.float32)
    lora_B = np.asarray(inputs["lora_B"], dtype=np.float32)
    sorted_ids = np.asarray(inputs["sorted_ids"]).astype(np.int64)
    row_slot = np.asarray(inputs["row_slot"]).astype(np.int64)
    slot_ranks = np.asarray(inputs["slot_ranks"]).astype(np.int64)

    # verify the runtime routing matches the compile-time layout
    expect_row_slot = np.full(BS, -1, dtype=np.int64)
    for s, (o, c) in enumerate(zip(SLOT_OFFS, SLOT_COUNTS)):
        expect_row_slot[o:o + c] = s
    if (not np.array_equal(row_slot, expect_row_slot)
            or not np.array_equal(slot_ranks, np.array(SLOT_RANKS))):
        return _host_reference(x, lora_A, lora_B, sorted_ids, row_slot, slot_ranks)

    trace = os.environ.get("LORA_TRACE", "0") == "1"
    if trace:
        _install_ntff_hook()

    reordered = x[sorted_ids]                      # [BS, H]

    # per-core token shares: core k takes rows off_s + [k*cs8, (k+1)*cs8)
    xt_maps = []
    core_rows = []
    for k in range(N_CORES):
        rows = np.concatenate([
            np.arange(SLOT_OFFS[s] + k * CS8[s], SLOT_OFFS[s] + (k + 1) * CS8[s])
            for s in range(S)
        ])
        core_rows.append(rows)
        rc = reordered[rows]                       # [448, 2048]
        # xt_host[p, k*448+t] = rc[t, k*128+p]
        xt_host = np.ascontiguousarray(
            rc.reshape(TCORE, KCH, 128).transpose(2, 1, 0)
        ).reshape(128, KCH * TCORE).astype(BF16)
        xt_maps.append(xt_host)

    # A pack: per pass a 128-col stationary block; swizzled so the DMA image
    # is [128, pass*16*128] with apk[p, pass*2048 + k*128 + c] =
    # A_pack[k*128+p, 128*pass + c]
    a_pack = np.zeros((H, len(PASSES) * 128), dtype=np.float32)
    for p, (s, mods) in enumerate(PASSES):
        r = SLOT_RANKS[s]
        for m, coff in mods:
            a_pack[:, p * 128 + coff: p * 128 + coff + r] = lora_A[m, s, :r, :].T
    apk_host = np.ascontiguousarray(
        a_pack.reshape(KCH, 128, len(PASSES), 128).transpose(1, 2, 0, 3)
    ).reshape(128, len(PASSES) * KCH * 128).astype(BF16)

    # B pack: column group g holds each (m,s) block at the partition range
    # matching its inter rows
    bpk_host = np.zeros((128, NBG * OUT), dtype=np.float32)
    for (m, s), g in BGRP.items():
        _, part = LOC[(m, s)]
        r = SLOT_RANKS[s]
        bpk_host[part:part + r, g * OUT:(g + 1) * OUT] = lora_B[m, s, :r, :]
    bpk_host = bpk_host.astype(BF16)

    from concourse.bass_utils import run_bass_kernel_spmd

    nc = _get_nc()
    in_maps = [
        {"xt": xt_maps[k], "apk": apk_host, "bpk": bpk_host}
        for k in range(N_CORES)
    ]
    res = run_bass_kernel_spmd(nc, in_maps, core_ids=list(range(N_CORES)), trace=trace)
    LAST_EXEC_NS = res.exec_time_ns
    if trace and res.instructions_and_trace:
        print(f"trace path: {res.instructions_and_trace[1]}", file=sys.stderr)

    out = np.zeros((BS, M * OUT), dtype=np.float32)
    for k in range(N_CORES):
        out[sorted_ids[core_rows[k]]] = res.results[k]["y"]
    return out


# revision 3
# speedup vs baseline: 1.3075x; 1.3075x over previous
"""Trainium2 Bass kernel for nn_LoraLayer (grouped LoRA GEMM with token routing).

Strategy: data-parallel over tokens with replicated (packed) LoRA weights.
Each of the 8 cores takes an equal 1/8 share of every slot's tokens
(128/192/96/32 = 448 tokens), so all cores run an identical program on
identically-shaped data.

Device kernel (per core):
  GEMM1: inter^T[r, t] = packed-A stationary passes (module-packed,
         partition offsets 32-aligned) x streamed x^T token columns,
         accumulated over 16 h-chunks of 128.
  GEMM2: out[t, o] = inter (stationary, partition range matching GEMM1
         output rows) x streamed B columns (B host-packed into the same
         partition ranges), N=512 per matmul.
  PSUM -> SBUF copies alternate VectorE / ScalarE (casting to bf16),
  output DMA per (token block, module); host upcasts to f32.

Host side: gather x rows by sorted_ids, build packed/swizzled weight
images, scatter per-core outputs back to token order.
"""

import contextlib
import ctypes
import os
import sys
import types

import ml_dtypes
import numpy as np

# ---------------------------------------------------------------- constants
BS, H, OUT = 4096, 2048, 2048
M, S, R = 3, 4, 64
SLOT_COUNTS = [1024, 1536, 768, 256]
SLOT_RANKS = [64, 32, 16, 48]
SLOT_OFFS = [0, 1024, 2560, 3328]
N_CORES = 8
CS8 = [c // N_CORES for c in SLOT_COUNTS]          # [128, 192, 96, 32]
TCORE = sum(CS8)                                   # 448
TOFF = [0, 128, 320, 416]                          # slot offsets within a core
KCH = H // 128                                     # 16 h-chunks

# GEMM1 passes: stationary A blocks.  (slot, width, [(module, col_off)]).
# col_off is also the PSUM partition where the module's inter rows land.
PASSES = [
    (0, 128, [(0, 0), (1, 64)]),
    (0, 64, [(2, 0)]),
    (1, 96, [(0, 0), (1, 32), (2, 64)]),
    (2, 80, [(0, 0), (1, 32), (2, 64)]),
    (3, 112, [(0, 0), (1, 64)]),
    (3, 48, [(2, 0)]),
]
PASS_OFF = [0]
for _s, _w, _m in PASSES:
    PASS_OFF.append(PASS_OFF[-1] + _w)
AW = PASS_OFF[-1]                                   # 528 total A columns
# (m, s) -> (pass_idx, partition_off)
LOC = {}
for _p, (_s, _w, _mods) in enumerate(PASSES):
    for _m, _c in _mods:
        LOC[(_m, _s)] = (_p, _c)
# (m, s) -> B column group (groups of 2048 cols; partition ranges within a
# group are disjoint and equal to LOC partition ranges)
BGRP = {(0, 0): 0, (1, 0): 0, (2, 0): 1, (1, 3): 1,
        (0, 1): 2, (1, 1): 2, (2, 1): 2,
        (0, 2): 3, (1, 2): 3, (2, 2): 3,
        (0, 3): 4, (2, 3): 5}
NBG = 6
# token blocks (slot, tok_off_in_core, T_b) -- PSUM partition blocks <=128
TBLOCKS = [(0, 0, 128), (1, 128, 128), (1, 256, 64), (2, 320, 96), (3, 416, 32)]

BF16 = ml_dtypes.bfloat16

LAST_EXEC_NS = None

_nc_cache = {}


def _install_ntff_hook():
    """Agent image's antenv lacks axon_hooks; recreate it so trace=True works."""
    if "antenv.axon_hooks" in sys.modules:
        return True
    so_path = "/opt/axon/libaxon_pjrt.so"
    try:
        lib = ctypes.CDLL(so_path)
    except OSError:
        return False
    if not hasattr(lib, "axon_start_nrt_profile"):
        return False
    lib.axon_start_nrt_profile.argtypes = [ctypes.POINTER(ctypes.c_int64), ctypes.c_size_t]
    lib.axon_start_nrt_profile.restype = ctypes.c_int64
    lib.axon_stop_nrt_profile.argtypes = [ctypes.c_char_p]
    lib.axon_stop_nrt_profile.restype = ctypes.c_int64

    @contextlib.contextmanager
    def _hook(output_dir, device_ids):
        import jax
        jax.devices()
        if device_ids:
            ids = (ctypes.c_int64 * len(device_ids))(*device_ids)
            rc = lib.axon_start_nrt_profile(ids, len(device_ids))
        else:
            rc = lib.axon_start_nrt_profile(None, 0)
        if rc != 0:
            raise RuntimeError(f"axon_start_nrt_profile rc={rc}")
        try:
            yield
        finally:
            n = lib.axon_stop_nrt_profile(str(output_dir).encode())
            print(f"ntff profile: {n} file(s) -> {output_dir}", file=sys.stderr)

    mod = types.ModuleType("antenv.axon_hooks")
    mod.get_axon_ntff_profile_hook = lambda: _hook
    mod.set_axon_ntff_profile_hook = lambda h: None
    sys.modules["antenv.axon_hooks"] = mod
    return True


def _build_nc():
    import concourse.bass as bass  # noqa: F401
    import concourse.mybir as mybir
    from concourse import bacc
    from concourse.tile import TileContext

    dt_in = mybir.dt.bfloat16

    nc = bacc.Bacc("TRN2", target_bir_lowering=False)
    xt = nc.dram_tensor("xt", [128, KCH * TCORE], dt_in, kind="ExternalInput")
    apk = nc.dram_tensor("apk", [128, KCH * AW], dt_in, kind="ExternalInput")
    bpk = nc.dram_tensor("bpk", [128, NBG * OUT], dt_in, kind="ExternalInput")
    y = nc.dram_tensor("y", [TCORE, M * OUT], dt_in, kind="ExternalOutput")

    with TileContext(nc) as tc:
        with (
            tc.tile_pool(name="w", bufs=1) as wpool,
            tc.tile_pool(name="interp", bufs=1) as ipool,
            tc.tile_pool(name="outp", bufs=4) as opool,
            tc.tile_pool(name="ps1", bufs=3, space="PSUM") as ps1,
            tc.tile_pool(name="ps2", bufs=4, space="PSUM") as ps2,
        ):
            # separate tiles per DMA unit so reads only wait for their slice
            xt_sb = [wpool.tile([128, 4 * TCORE], dt_in, tag=f"xt{q}", name=f"xt_sb{q}") for q in range(4)]
            a_sb = [wpool.tile([128, KCH * PASSES[p][1]], dt_in, tag=f"a{p}", name=f"a_sb{p}")
                    for p in range(len(PASSES))]
            b_sb = [wpool.tile([128, OUT], dt_in, tag=f"b{g}", name=f"b_sb{g}") for g in range(NBG)]

            def dma_xt(q):
                nc.sync.dma_start(xt_sb[q][:, :], xt[:, q * 4 * TCORE:(q + 1) * 4 * TCORE])

            def dma_a(p):
                w = PASSES[p][1]
                nc.sync.dma_start(a_sb[p][:, :], apk[:, PASS_OFF[p] * KCH: PASS_OFF[p] * KCH + w * KCH])

            def dma_b(g):
                nc.sync.dma_start(b_sb[g][:, :], bpk[:, g * OUT:(g + 1) * OUT])

            for q in range(4):
                dma_xt(q)
            dma_a(0)
            dma_a(1)
            dma_b(0)
            dma_b(1)
            dma_a(2)
            dma_b(2)
            dma_a(3)
            dma_b(3)
            dma_a(4)
            dma_a(5)
            dma_b(4)
            dma_b(5)

            inter = [None] * len(PASSES)
            copy_i = 0

            def gemm1(p):
                s, w, _mods = PASSES[p]
                T = CS8[s]
                f32 = mybir.dt.float32
                pt = ps1.tile([128, T], f32, tag="ps1")
                for k in range(KCH):
                    nc.tensor.matmul(
                        pt[:w, :T],
                        lhsT=a_sb[p][:, k * w:(k + 1) * w],
                        rhs=xt_sb[k // 4][:, (k % 4) * TCORE + TOFF[s]:
                                          (k % 4) * TCORE + TOFF[s] + T],
                        start=(k == 0),
                        stop=(k == KCH - 1),
                    )
                it = ipool.tile([128, T], dt_in, tag=f"inter{p}")
                nc.vector.tensor_copy(it[:w, :], pt[:w, :T])
                inter[p] = it

            def gemm2(tb):
                nonlocal copy_i
                s, toff, Tb = tb
                r = SLOT_RANKS[s]
                f32 = mybir.dt.float32
                ot = opool.tile([128, M * OUT], dt_in, tag="out")
                for m in range(M):
                    p, part = LOC[(m, s)]
                    g = BGRP[(m, s)]
                    lhs = inter[p][part:part + r, toff - TOFF[s]: toff - TOFF[s] + Tb]
                    for oc in range(4):
                        pt = ps2.tile([128, 512], f32, tag="ps2")
                        nc.tensor.matmul(
                            pt[:Tb, :],
                            lhsT=lhs,
                            rhs=b_sb[g][part:part + r, oc * 512:(oc + 1) * 512],
                            start=True,
                            stop=True,
                        )
                        dst = ot[:Tb, m * OUT + oc * 512: m * OUT + (oc + 1) * 512]
                        if copy_i % 2 == 0:
                            nc.vector.tensor_copy(dst, pt[:Tb, :])
                        else:
                            nc.scalar.copy(dst, pt[:Tb, :])
                        copy_i += 1
                    nc.sync.dma_start(
                        y[toff:toff + Tb, m * OUT:(m + 1) * OUT],
                        ot[:Tb, m * OUT:(m + 1) * OUT],
                    )

            # interleave: slot's GEMM2 as soon as its passes are done
            gemm1(0)
            gemm1(1)
            gemm2(TBLOCKS[0])
            gemm1(2)
            gemm2(TBLOCKS[1])
            gemm2(TBLOCKS[2])
            gemm1(3)
            gemm2(TBLOCKS[3])
            gemm1(4)
            gemm1(5)
            gemm2(TBLOCKS[4])

    nc.finalize()
    return nc


def _get_nc():
    if "nc" not in _nc_cache:
        _nc_cache["nc"] = _build_nc()
    return _nc_cache["nc"]


def _host_reference(x, lora_A, lora_B, sorted_ids, row_slot, slot_ranks):
    """Numpy fallback (used only if runtime slot structure deviates from the
    compile-time layout)."""
    reordered = x[sorted_ids]
    out = np.zeros((BS, M * OUT), dtype=np.float32)
    for b in range(BS):
        s = row_slot[b]
        if s < 0:
            continue
        r = int(slot_ranks[s])
        row = reordered[b]
        for m in range(M):
            inter = row @ lora_A[m, s, :r, :].T
            out[sorted_ids[b], m * OUT:(m + 1) * OUT] = inter @ lora_B[m, s, :r, :]
    return out


def kernel(**inputs):
    global LAST_EXEC_NS
    x = np.asarray(inputs["x"], dtype=np.float32)
    lora_A = np.asarray(inputs["lora_A"], dtype=np.float32)
    lora_B = np.asarray(inputs["lora_B"], dtype=np.float32)
    sorted_ids = np.asarray(inputs["sorted_ids"]).astype(np.int64)
    row_slot = np.asarray(inputs["row_slot"]).astype(np.int64)
    slot_ranks = np.asarray(inputs["slot_ranks"]).astype(np.int64)

    # verify the runtime routing matches the compile-time layout
    expect_row_slot = np.full(BS, -1, dtype=np.int64)
    for s, (o, c) in enumerate(zip(SLOT_OFFS, SLOT_COUNTS)):
        expect_row_slot[o:o + c] = s
    if (not np.array_equal(row_slot, expect_row_slot)
            or not np.array_equal(slot_ranks, np.array(SLOT_RANKS))):
        return _host_reference(x, lora_A, lora_B, sorted_ids, row_slot, slot_ranks)

    trace = os.environ.get("LORA_TRACE", "0") == "1"
    if trace:
        _install_ntff_hook()

    reordered = x[sorted_ids]                      # [BS, H]

    # per-core token shares: core k takes rows off_s + [k*cs8, (k+1)*cs8)
    xt_maps = []
    core_rows = []
    for k in range(N_CORES):
        rows = np.concatenate([
            np.arange(SLOT_OFFS[s] + k * CS8[s], SLOT_OFFS[s] + (k + 1) * CS8[s])
            for s in range(S)
        ])
        core_rows.append(rows)
        rc = reordered[rows]                       # [448, 2048]
        # xt_host[p, k*448+t] = rc[t, k*128+p]
        xt_host = np.ascontiguousarray(
            rc.reshape(TCORE, KCH, 128).transpose(2, 1, 0)
        ).reshape(128, KCH * TCORE).astype(BF16)
        xt_maps.append(xt_host)

    # A pack, swizzled: apk[p, PASS_OFF[pass]*16 + k*w + c] =
    #   A^T[k*128+p, PASS_OFF[pass] + c]
    a_pack = np.zeros((H, AW), dtype=np.float32)
    for p, (s, w, mods) in enumerate(PASSES):
        r = SLOT_RANKS[s]
        for m, coff in mods:
            a_pack[:, PASS_OFF[p] + coff: PASS_OFF[p] + coff + r] = lora_A[m, s, :r, :].T
    cols = []
    for p, (s, w, mods) in enumerate(PASSES):
        blk = a_pack[:, PASS_OFF[p]:PASS_OFF[p] + w]      # [2048, w]
        cols.append(blk.reshape(KCH, 128, w).transpose(1, 0, 2).reshape(128, KCH * w))
    apk_host = np.ascontiguousarray(np.concatenate(cols, axis=1)).astype(BF16)

    # B pack: column group g holds each (m,s) block at the partition range
    # matching its inter rows
    bpk_host = np.zeros((128, NBG * OUT), dtype=np.float32)
    for (m, s), g in BGRP.items():
        _, part = LOC[(m, s)]
        r = SLOT_RANKS[s]
        bpk_host[part:part + r, g * OUT:(g + 1) * OUT] = lora_B[m, s, :r, :]
    bpk_host = bpk_host.astype(BF16)

    from concourse.bass_utils import run_bass_kernel_spmd

    nc = _get_nc()
    in_maps = [
        {"xt": xt_maps[k], "apk": apk_host, "bpk": bpk_host}
        for k in range(N_CORES)
    ]
    res = run_bass_kernel_spmd(nc, in_maps, core_ids=list(range(N_CORES)), trace=trace)
    LAST_EXEC_NS = res.exec_time_ns
    if trace and res.instructions_and_trace:
        print(f"trace path: {res.instructions_and_trace[1]}", file=sys.stderr)

    out = np.zeros((BS, M * OUT), dtype=np.float32)
    for k in range(N_CORES):
        out[sorted_ids[core_rows[k]]] = res.results[k]["y"].astype(np.float32)
    return out
